# revision 1
# baseline (speedup 1.0000x reference)
"""2-layer LSTM (T=128, B=256, V=256, E=512, NN=1024) on 8 TRN2 NeuronCores.

Strategy: tensor-parallel over the gate/hidden dimension (each core owns 128
h-rows of each layer = 512 gate rows), batch kept whole (moving dim N=256 so
float32r matmuls run at full PE rate). Everything on device is transposed
[features, batch] so the recurrence needs no transposes. The embedding matmul
is folded into the layer-0 input weights (M0 = emb @ W0[:E]) so the x-path
contracts directly over the vocab. One merged AllGather per step carries
[h0(t+2) | h1(t)] (skew-2 so layer-1 h0-matmuls overlap the in-flight AG). The output projection is split by
vocab columns across cores (32 each) and interleaved as PE filler; bias rows
ride the matmuls via a ones-row trick.
"""

from contextlib import ExitStack

import numpy as np

F32 = None
F32R = None

T, B, V, E, NN = 128, 256, 256, 512, 1024
NCORES = 8
GS = 128            # rows per gate per core
VS = V // NCORES    # output vocab columns per core
KC_U = V // 128     # u chunks (contraction over vocab)
KC_H = NN // 128    # h chunks

_CACHE = {}


def _build():
    import concourse.tile as tile
    from concourse import bacc, mybir

    global F32, F32R
    F32 = mybir.dt.float32
    F32R = mybir.dt.float32r

    nc = bacc.Bacc("TRN2", target_bir_lowering=False, debug=False,
                   num_devices=NCORES)

    u_T = nc.dram_tensor("u_T", [T, V, B], F32R, kind="ExternalInput")
    m0 = nc.dram_tensor("m0", [KC_U, 128, 4, GS], F32R, kind="ExternalInput")
    w0h = nc.dram_tensor("w0h", [KC_H, 128, 4, GS], F32R, kind="ExternalInput")
    w1h0 = nc.dram_tensor("w1h0", [KC_H, 128, 4, GS], F32R, kind="ExternalInput")
    w1h1 = nc.dram_tensor("w1h1", [KC_H, 128, 4, GS], F32R, kind="ExternalInput")
    wout = nc.dram_tensor("wout", [KC_H, 128, VS], F32R, kind="ExternalInput")
    b0 = nc.dram_tensor("b0", [128, 4], F32, kind="ExternalInput")
    b1 = nc.dram_tensor("b1", [128, 4], F32, kind="ExternalInput")
    bout = nc.dram_tensor("bout", [1, VS], F32R, kind="ExternalInput")
    ones = nc.dram_tensor("ones", [1, B], F32R, kind="ExternalInput")
    zero_h = nc.dram_tensor("zero_h", [128, B], F32R, kind="ExternalInput")
    logits = nc.dram_tensor("logits", [T, VS, B], F32, kind="ExternalOutput")

    with tile.TileContext(nc) as tc, ExitStack() as ctx:
        wp = ctx.enter_context(tc.tile_pool(name="wp", bufs=1))
        state = ctx.enter_context(tc.tile_pool(name="state", bufs=1))
        hbuf = ctx.enter_context(tc.tile_pool(name="hbuf", bufs=3))
        act = ctx.enter_context(tc.tile_pool(name="act", bufs=2))
        ups = ctx.enter_context(tc.tile_pool(name="ups", bufs=3))
        pay = ctx.enter_context(tc.tile_pool(name="pay", bufs=2))
        outp = ctx.enter_context(tc.tile_pool(name="outp", bufs=3))
        ps = ctx.enter_context(tc.tile_pool(name="ps", bufs=1, space="PSUM"))
        dram = ctx.enter_context(tc.tile_pool(name="dram", bufs=3, space="DRAM"))

        m0_t = wp.tile([128, KC_U, 4, GS], F32R)
        w0h_t = wp.tile([128, KC_H, 4, GS], F32R)
        w1h0_t = wp.tile([128, KC_H, 4, GS], F32R)
        w1h1_t = wp.tile([128, KC_H, 4, GS], F32R)
        wout_t = wp.tile([128, KC_H, VS], F32R)
        b0_t = wp.tile([128, 4], F32)
        b1_t = wp.tile([128, 4], F32)
        bout_t = wp.tile([1, VS], F32R)
        ones_t = wp.tile([1, B], F32R)
        for dst, src in [(m0_t, m0), (w0h_t, w0h), (w1h0_t, w1h0),
                         (w1h1_t, w1h1)]:
            nc.sync.dma_start(dst[:], src[:].rearrange("k p a g -> p k a g"))
        nc.sync.dma_start(wout_t[:], wout[:].rearrange("k p g -> p k g"))
        nc.sync.dma_start(b0_t[:], b0[:])
        nc.sync.dma_start(b1_t[:], b1[:])
        nc.sync.dma_start(bout_t[:], bout[:])
        nc.sync.dma_start(ones_t[:], ones[:])

        c0_t = state.tile([128, B], F32)
        c1_t = state.tile([128, B], F32)
        nc.gpsimd.memset(c0_t[:], 0.0)
        nc.gpsimd.memset(c1_t[:], 0.0)

        sig = mybir.ActivationFunctionType.Sigmoid
        tanh = mybir.ActivationFunctionType.Tanh

        def cell(layer, gates_ps, c_t, b_t, h_out):
            f_t = act.tile([128, B], F32, tag=f"f{layer}")
            i_t = act.tile([128, B], F32, tag=f"i{layer}")
            o_t = act.tile([128, B], F32, tag=f"o{layer}")
            g_t = act.tile([128, B], F32, tag=f"g{layer}")
            nc.scalar.activation(f_t[:], gates_ps[:, 0, :], sig, bias=b_t[:, 0:1])
            nc.scalar.activation(i_t[:], gates_ps[:, 1, :], sig, bias=b_t[:, 1:2])
            nc.scalar.activation(o_t[:], gates_ps[:, 2, :], sig, bias=b_t[:, 2:3])
            nc.scalar.activation(g_t[:], gates_ps[:, 3, :], tanh, bias=b_t[:, 3:4])
            ig_t = act.tile([128, B], F32, tag=f"ig{layer}")
            nc.vector.tensor_mul(ig_t[:], i_t[:], g_t[:])
            nc.vector.tensor_mul(c_t[:], f_t[:], c_t[:])
            nc.vector.tensor_add(c_t[:], c_t[:], ig_t[:])
            tc_t = act.tile([128, B], F32, tag=f"tc{layer}")
            nc.scalar.activation(tc_t[:], c_t[:], tanh)
            nc.vector.tensor_mul(h_out, o_t[:], tc_t[:])

        def layer0(tau, h0f):
            ut = ups.tile([128, KC_U, B], F32R, tag="ut")
            nc.sync.dma_start(ut[:], u_T[tau].rearrange("(k p) n -> p k n", p=128))
            g0 = ps.tile([128, 4, B], F32, tag="g0ps")
            for gi in range(4):
                for k in range(KC_U):
                    nc.tensor.matmul(g0[:, gi, :], m0_t[:, k, gi, :],
                                     ut[:, k, :], start=(k == 0), stop=False)
                for k in range(KC_H):
                    nc.tensor.matmul(g0[:, gi, :], w0h_t[:, k, gi, :],
                                     h0f[:, k, :],
                                     start=False, stop=(k == KC_H - 1))
            return g0

        def layer0_first(tau):
            ut = ups.tile([128, KC_U, B], F32R, tag="ut")
            nc.sync.dma_start(ut[:], u_T[tau].rearrange("(k p) n -> p k n", p=128))
            g0 = ps.tile([128, 4, B], F32, tag="g0ps")
            for gi in range(4):
                for k in range(KC_U):
                    nc.tensor.matmul(g0[:, gi, :], m0_t[:, k, gi, :],
                                     ut[:, k, :], start=(k == 0),
                                     stop=(k == KC_U - 1))
            return g0

        def merged_ag(pay_t):
            bnc = dram.tile([128, 2 * B], F32R, tag="bnc")
            nc.sync.dma_start(bnc[:], pay_t[:])
            gath = dram.tile([NCORES * 128, 2 * B], F32R, tag="gath",
                             addr_space="Shared")
            nc.gpsimd.collective_compute(
                "AllGather", mybir.AluOpType.bypass,
                replica_groups=[list(range(NCORES))],
                ins=[bnc[:].opt()], outs=[gath[:].opt()],
            )
            gv = gath[:].rearrange("(k p) n -> p k n", p=128)
            h0f = hbuf.tile([128, KC_H, B], F32R, tag="h0f")
            nc.sync.dma_start(h0f[:], gv[:, :, 0:B])
            h1f = hbuf.tile([128, KC_H, B], F32R, tag="h1f")
            nc.gpsimd.dma_start(h1f[:], gv[:, :, B:2 * B])
            return h0f, h1f

        def outproj(t, h1f):
            lg = ps.tile([VS, B], F32, tag="lgps", bufs=2)
            for k in range(KC_H):
                nc.tensor.matmul(lg[:], wout_t[:, k, :], h1f[:, k, :],
                                 start=(k == 0), stop=False)
            nc.tensor.matmul(lg[:], bout_t[:], ones_t[:], start=False, stop=True)
            lo = outp.tile([VS, B], F32, tag="lo")
            nc.vector.tensor_copy(lo[:], lg[:])
            nc.sync.dma_start(logits[t], lo[:])

        # pre-loop: seed h0_full(0) and h0_full(1) via two AGs
        pay_t = pay.tile([128, 2 * B], F32R, tag="pay")
        nc.sync.dma_start(pay_t[:, B:2 * B], zero_h[:])
        g0 = layer0_first(0)
        cell(0, g0, c0_t, b0_t, pay_t[:, 0:B])
        h0A, h1_full = merged_ag(pay_t)       # h0_full(0), h1(-1)=0
        pay_t = pay.tile([128, 2 * B], F32R, tag="pay")
        nc.sync.dma_start(pay_t[:, B:2 * B], zero_h[:])
        g0 = layer0(1, h0A)
        cell(0, g0, c0_t, b0_t, pay_t[:, 0:B])
        h0B, _h1z = merged_ag(pay_t)          # h0_full(1)

        # skew-2 steady state: AG(tau) carries [h0(tau+2) | h1(tau)].
        # g1's w1h0 part reads the two-generations-old gather, so the
        # scheduler runs it while the latest AG is still in flight.
        for tau in range(T):
            g1 = ps.tile([128, 4, B], F32, tag="g1ps")
            for gi in range(4):
                for k in range(KC_H):
                    nc.tensor.matmul(g1[:, gi, :], w1h0_t[:, k, gi, :],
                                     h0A[:, k, :], start=(k == 0), stop=False)
                for k in range(KC_H):
                    nc.tensor.matmul(g1[:, gi, :], w1h1_t[:, k, gi, :],
                                     h1_full[:, k, :],
                                     start=False, stop=(k == KC_H - 1))
            if tau > 0:
                outproj(tau - 1, h1_full)

            pay_t = pay.tile([128, 2 * B], F32R, tag="pay")
            cell(1, g1, c1_t, b1_t, pay_t[:, B:2 * B])

            if tau + 2 < T:
                g0 = layer0(tau + 2, h0B)
                cell(0, g0, c0_t, b0_t, pay_t[:, 0:B])
            else:
                nc.sync.dma_start(pay_t[:, 0:B], zero_h[:])

            h0_new, h1_full = merged_ag(pay_t)
            h0A, h0B = h0B, h0_new

        outproj(T - 1, h1_full)

    nc.compile()
    return nc


def _host_inputs(inputs, emb, W0, b0, W1, b1, Wout, bout):
    f32 = np.float32
    M0 = emb.astype(f32) @ W0[:E].astype(f32)  # embedding folded into layer 0
    u_T = np.ascontiguousarray(inputs.transpose(0, 2, 1)).astype(f32)
    in_maps = []
    for k in range(NCORES):
        rows = slice(128 * k, 128 * (k + 1))
        cols = np.concatenate([np.arange(g * NN, g * NN + NN)[rows]
                               for g in range(4)])
        in_maps.append({
            "u_T": u_T,
            "m0": np.ascontiguousarray(
                M0[:, cols].reshape(KC_U, 128, 4, GS)).astype(f32),
            "w0h": np.ascontiguousarray(
                W0[E:, cols].reshape(KC_H, 128, 4, GS)).astype(f32),
            "w1h0": np.ascontiguousarray(
                W1[:NN, cols].reshape(KC_H, 128, 4, GS)).astype(f32),
            "w1h1": np.ascontiguousarray(
                W1[NN:, cols].reshape(KC_H, 128, 4, GS)).astype(f32),
            "wout": np.ascontiguousarray(
                Wout[:, VS * k:VS * (k + 1)].reshape(KC_H, 128, VS)).astype(f32),
            "b0": np.ascontiguousarray(b0[cols].reshape(4, GS).T).astype(f32),
            "b1": np.ascontiguousarray(b1[cols].reshape(4, GS).T).astype(f32),
            "bout": bout[VS * k:VS * (k + 1)].reshape(1, VS).astype(f32),
            "ones": np.ones((1, B), f32),
            "zero_h": np.zeros((128, B), f32),
        })
    return in_maps


def _assemble(results):
    lg = np.concatenate([results[k]["logits"] for k in range(NCORES)], axis=1)
    return np.ascontiguousarray(lg.transpose(0, 2, 1)).reshape(T * B, V)


def kernel(inputs, emb, W0, b0, W1, b1, Wout, bout):
    from concourse import bass_utils

    inputs = np.asarray(inputs)
    if "nc" not in _CACHE:
        _CACHE["nc"] = _build()
    nc = _CACHE["nc"]
    in_maps = _host_inputs(np.asarray(inputs), np.asarray(emb), np.asarray(W0),
                           np.asarray(b0), np.asarray(W1), np.asarray(b1),
                           np.asarray(Wout), np.asarray(bout))
    res = bass_utils.run_bass_kernel_spmd(nc, in_maps,
                                          core_ids=list(range(NCORES)))
    out = _assemble(res.results)
    return out.astype(np.float32)

